# revision 38
# baseline (speedup 1.0000x reference)
"""Sparse PIoU (pixel-wise IoU) pairwise matrix kernel for Trainium2, 8 cores.

Math: for each pair (pred box n, target box m) the reference samples a 16x16
grid of the joint AABB and evaluates soft memberships
F = sigmoid(k(w/2-|A|)) * sigmoid(k(h/2-|B|)) per box, with (A, B) the pixel
offsets rotated into the box frame.  A and B are affine in the grid coords
(ug, uh), so the sigmoid args P = s/2 - A and Q = s/2 + A for all 256 pixels
x 4 fields come from K=12 matmuls against a tiny constant basis (1, ug, uh
per field); since P + Q = s >= 8 and k = 10, sigmoid(kP)*sigmoid(kQ) ==
sigmoid(k*min(P,Q)) to machine precision, giving the |.| for free.

Sparsity: boxes are small (8..96 px) in a 640x640 field, so only ~7% of the
512x512 pairs have margin-dilated AABB overlap AND pass a 2px-margin
separating-axis test; every excluded pair has true PIoU < 1e-12 (sigmoid
tails at >= 2px separation).  The host computes both filters, round-robins
surviving pairs over the 8 cores packed one-per-partition into units of 128
(~17 units/core vs the dense kernel's 256), and scatters device results
into a zero matrix.  The sub-unit tail spill (< 128 pairs/core) is computed
exactly on the host so the device runs one unit fewer.

Device pipeline per unit u (128 pairs):
    PE  : 4 matmuls [12,128]x[12,512] -> PSUM [128, 2048] = P|Q sig args
          (P and Q are two weight-sets against the same shared basis)
    ACT : sig = Sigmoid(K * PQ)            [128, 2048] bf16 (one instruction)
    DVE : Gm  = sigP * sigQ                [128, 1024] (field memberships)
    DVE : Fp  = gA * gB,  accum -> S[u]    (fused product+reduce, STT)
    DVE : F12 = F1 * F2,  accum -> I[u]    (fused product+reduce, STT)
The raw S|I accumulators stream out by DMA (all-but-last unit early, last
unit at the end); the tiny piou = I/(S-I+eps) division happens on the host.
Measured: the ACT sigmoid stream (1.97us/unit, zero gaps) and the DVE chain
(1.97us/unit) are co-paced bottlenecks; PE units take 1.7us so matmuls stay
ahead; the last unit's sigmoid runs in two field-halves to shorten the
closing DVE chain.  ~24us of the span is fixed NEFF preamble/postamble.

Host precomputes the per-pair coefficient slab directly in the transposed
fp16 layout the PE wants (lhsT [24, pairs]), eliminating the on-device
coefficient phase, PE transposes and stash copies of the dense kernel.  The
unit-0 coefficients ride in one combined tensor with the basis so a single
early DMA unblocks the first matmuls.

Dispatch uses a persistent jitted shard_map callable (cached per unit-count
U, recompiled automatically if denser inputs need more units) so
steady-state calls skip jax re-trace/re-lowering.
"""

import numpy as np

N = 512
M = 512
G = 16
NPIX = G * G
K_SLOPE = np.float32(10.0)
EPS = np.float32(1e-6)
NC = 8
DELTA = np.float32(2.0)  # separation margin in px (excluded-pair PIoU < 1e-12)

_cache = {}


def _derived(b):
    # b: [K,5] float32 -> per-box derived quantities (all float32)
    cx, cy, w, h, t = (b[:, i].astype(np.float32) for i in range(5))
    c, s = np.cos(t).astype(np.float32), np.sin(t).astype(np.float32)
    hw = np.float32(0.5) * (w * np.abs(c) + h * np.abs(s))
    hh = np.float32(0.5) * (w * np.abs(s) + h * np.abs(c))
    return dict(
        cx=cx, cy=cy, ct=c, st=s,
        shw=np.float32(0.5) * w, shh=np.float32(0.5) * h,
        x0=cx - hw, x1=cx + hw, y0=cy - hh, y1=cy + hh,
    )


def _basis():
    # [12, 1024] fp16 (values exact): field f at cols f*256..(f+1)*256 uses
    # rows 3f..3f+2 = (1, Ug, Uh).  Pixel p = h*G+g -> Ug[p]=u[g], Uh[p]=u[h].
    # The P and Q sigmoid-arg blocks share this basis (their coefficients are
    # two weight-sets against the same moving tensor).
    u = (np.arange(G, dtype=np.float32) + np.float32(0.5)) / np.float32(G)
    Ug = np.tile(u, G)
    Uh = np.repeat(u, G)
    bas = np.zeros((12, 4 * NPIX), dtype=np.float32)
    for f in range(4):
        c0 = f * NPIX
        bas[3 * f + 0, c0:c0 + NPIX] = 1.0
        bas[3 * f + 1, c0:c0 + NPIX] = Ug
        bas[3 * f + 2, c0:c0 + NPIX] = Uh
    return bas.astype(np.float16)  # (2g+1)/32 values are exact in fp16


def _sat_separated(P, T, n_idx, m_idx, margin):
    """True for pairs whose margin-dilated rotated boxes are disjoint
    (separating-axis test on the 4 edge normals).  At k=10, a separation
    margin of 2px bounds the true PIoU of excluded pairs below ~1e-12."""
    dcx = T["cx"][m_idx] - P["cx"][n_idx]
    dcy = T["cy"][m_idx] - P["cy"][n_idx]
    sep = np.zeros(n_idx.size, dtype=bool)
    for src in (0, 1):
        B1, i1 = (P, n_idx) if src == 0 else (T, m_idx)
        B2, i2 = (T, m_idx) if src == 0 else (P, n_idx)
        ct, st = B1["ct"][i1], B1["st"][i1]
        c2, s2 = B2["ct"][i2], B2["st"][i2]
        for ax in range(2):
            ux, uy = (ct, st) if ax == 0 else (-st, ct)
            e1 = B1["shw" if ax == 0 else "shh"][i1]
            e2 = (B2["shw"][i2] * np.abs(ux * c2 + uy * s2)
                  + B2["shh"][i2] * np.abs(-ux * s2 + uy * c2))
            sep |= np.abs(ux * dcx + uy * dcy) > e1 + e2 + margin
    return sep


def _host_exact(loc_p, loc_t, pairs):
    """Exact reference math (vectorized numpy) for a small list of pairs —
    used for the tail spill so the device runs one unit fewer per core."""
    b1 = loc_p[pairs[:, 0]]
    b2 = loc_t[pairs[:, 1]]
    u = (np.arange(G, dtype=np.float32) + np.float32(0.5)) / np.float32(G)

    def aabb(b):
        cx, cy, w, h, t = (b[:, i] for i in range(5))
        c, s = np.abs(np.cos(t)), np.abs(np.sin(t))
        hw = 0.5 * (w * c + h * s)
        hh = 0.5 * (w * s + h * c)
        return cx - hw, cx + hw, cy - hh, cy + hh

    x0p, x1p, y0p, y1p = aabb(b1)
    x0t, x1t, y0t, y1t = aabb(b2)
    xmin = np.minimum(x0p, x0t)
    xmax = np.maximum(x1p, x1t)
    ymin = np.minimum(y0p, y0t)
    ymax = np.maximum(y1p, y1t)
    px = xmin[:, None] + (xmax - xmin)[:, None] * u[None, :]  # [k,G]
    py = ymin[:, None] + (ymax - ymin)[:, None] * u[None, :]
    PX = px[:, None, :]  # [k,1,G] broadcast over rows
    PY = py[:, :, None]  # [k,G,1]

    def member(b):
        cx, cy, w, h, t = (b[:, i, None, None] for i in range(5))
        ct, st = np.cos(t), np.sin(t)
        dx = PX - cx
        dy = PY - cy
        dw = np.abs(dx * ct + dy * st)
        dh = np.abs(-dx * st + dy * ct)
        def sig(z):
            with np.errstate(over="ignore"):  # exp overflow -> sig == 0.0
                return 1.0 / (1.0 + np.exp(-z))

        return (sig(float(K_SLOPE) * (0.5 * w - dw))
                * sig(float(K_SLOPE) * (0.5 * h - dh)))

    F1 = member(b1)
    F2 = member(b2)
    inter = np.sum(F1 * F2, axis=(1, 2))
    union = np.sum(F1, axis=(1, 2)) + np.sum(F2, axis=(1, 2)) - inter
    return (inter / (union + EPS)).astype(np.float32)


def _pair_coeffs(P, T, n_idx, m_idx):
    """[24, npairs] float32 coefficient slab for the given (n, m) pairs.

    Row 3f+r holds the P-arg coeff of field f on basis fn r in (1, ug, uh);
    row 12+3f+r the Q-arg coeff.  Field order: A-pred, A-targ, B-pred, B-targ
    so that Gm pairs A1|A2 with B1|B2 and Fp pairs F1 with F2 downstream.
    """
    p = {k: v[n_idx] for k, v in P.items()}
    t = {k: v[m_idx] for k, v in T.items()}
    xmin = np.minimum(p["x0"], t["x0"])
    xmax = np.maximum(p["x1"], t["x1"])
    ymin = np.minimum(p["y0"], t["y0"])
    ymax = np.maximum(p["y1"], t["y1"])
    sx = xmax - xmin
    sy = ymax - ymin
    C = np.empty((24, n_idx.size), dtype=np.float32)
    for f, (b, ab) in enumerate(((p, "a"), (t, "a"), (p, "b"), (t, "b"))):
        dx0 = xmin - b["cx"]
        dy0 = ymin - b["cy"]
        if ab == "a":
            c0 = dx0 * b["ct"] + dy0 * b["st"]
            c1 = sx * b["ct"]
            c2 = sy * b["st"]
            half = b["shw"]
        else:
            c0 = dy0 * b["ct"] - dx0 * b["st"]
            c1 = -sx * b["st"]
            c2 = sy * b["ct"]
            half = b["shh"]
        C[3 * f + 0] = half - c0
        C[3 * f + 1] = -c1
        C[3 * f + 2] = -c2
        C[12 + 3 * f + 0] = half + c0
        C[12 + 3 * f + 1] = c1
        C[12 + 3 * f + 2] = c2
    return C


def _build_nc(U):
    from contextlib import ExitStack

    import concourse.bacc as bacc
    import concourse.tile as tile
    from concourse import mybir

    dt = mybir.dt
    op = mybir.AluOpType
    AF = mybir.ActivationFunctionType
    K = float(K_SLOPE)

    nc = bacc.Bacc(None, target_bir_lowering=False)
    # FT packs [unit-0 P-coeffs | unit-0 Q-coeffs | basis] so one fast DMA
    # delivers everything the first unit's matmuls need.
    FT_d = nc.declare_dram_parameter("FT", [12, 256 + 4 * NPIX], dt.float16, isOutput=False)
    UR = max(U - 1, 1)
    LHPR_d = nc.declare_dram_parameter("LHPR", [12, UR * 128], dt.float16, isOutput=False)
    LHQR_d = nc.declare_dram_parameter("LHQR", [12, UR * 128], dt.float16, isOutput=False)
    # Raw S|I accumulators; the tiny piou = I/(S-I+eps) division happens on
    # the host, which keeps the device tail to just the last STT + DMA.
    OUT_d = nc.declare_dram_parameter("OUT", [128, 2 * U], dt.float32, isOutput=True)

    with tile.TileContext(nc) as tc, ExitStack() as ctx:
        consts = ctx.enter_context(tc.tile_pool(name="consts", bufs=1))
        sigp = ctx.enter_context(tc.tile_pool(name="sigp", bufs=6))
        gmp = ctx.enter_context(tc.tile_pool(name="gmp", bufs=4))
        fpp = ctx.enter_context(tc.tile_pool(name="fpp", bufs=4))
        accp = ctx.enter_context(tc.tile_pool(name="accp", bufs=1))
        psum = ctx.enter_context(tc.tile_pool(name="psum", bufs=2, space="PSUM"))

        # Warm the PE clock on a memset scratch tile: no DMA dependency, so
        # the ramp starts right after the preamble barrier.  The memset runs
        # on the (otherwise idle) vector queue so the gpsimd/sync queues get
        # their DMA triggers out without delay.
        Wz = consts.tile([12, 128], dt.bfloat16)
        nc.vector.memset(Wz[:], 0.0)
        Tw = psum.tile([128, 8 * NPIX], dt.float32, tag="pq")
        nc.tensor.matmul(
            Tw[:, 0:128], Wz[:], Wz[:], start=True, stop=True)

        # Input DMAs: all on the sync queue (hardware DGE; gpsimd triggers go
        # through the slow SWDGE path, and a scalar-queue trigger makes
        # walrus re-emit the sigmoid ACT_TABLE_LOAD).  FT (unit-0 coeffs +
        # basis, ~31KB) first so unit 0 starts ~1.5us before the bulk slabs
        # finish landing.
        FT = consts.tile([12, 256 + 4 * NPIX], dt.float16)
        LHPR = consts.tile([12, UR, 128], dt.float16)
        LHQR = consts.tile([12, UR, 128], dt.float16)
        nc.sync.dma_start(out=FT[:], in_=FT_d[:])
        nc.sync.dma_start(
            out=LHPR[:].rearrange("p a b -> p (a b)"), in_=LHPR_d[:])
        nc.sync.dma_start(
            out=LHQR[:].rearrange("p a b -> p (a b)"), in_=LHQR_d[:])
        BAS = FT[:, 256:256 + 4 * NPIX]

        SI = accp.tile([128, 2, U], dt.float32)
        Ssum = SI[:, 0, :]
        Isum = SI[:, 1, :]
        OUTv = OUT_d[:].rearrange("p (a b) -> p a b", a=2)

        # S|I columns DMA out in two chunks: all but the last unit while the
        # ACT stream is still running, the final column right at the end.
        U1 = U - 1 if U > 1 else U

        for u in range(U):
            PQ = psum.tile([128, 8 * NPIX], dt.float32, tag="pq")
            for h in range(4):
                if u == 0:
                    lhsT = FT[:, 0:128] if h < 2 else FT[:, 128:256]
                else:
                    lhsT = (LHPR if h < 2 else LHQR)[:, u - 1, :]
                nc.tensor.matmul(
                    PQ[:, h * 512:(h + 1) * 512],
                    lhsT,
                    BAS[:, (h % 2) * 512:(h % 2 + 1) * 512],
                    start=True, stop=True)
            last = u == U - 1 and U > 1
            if not last:
                sig = sigp.tile([128, 8 * NPIX], dt.bfloat16, tag="sig")
                nc.scalar.activation(sig[:], PQ[:], AF.Sigmoid, 0.0, K)
                Gm = gmp.tile([128, 4 * NPIX], dt.bfloat16, tag="Gm")
                nc.vector.tensor_tensor(
                    Gm[:], sig[:, 0:1024], sig[:, 1024:2048], op.mult)
            else:
                # Final unit: sigmoid in two field-halves (A then B) so the
                # closing DVE chain starts one ACT-half earlier.  PQ viewed
                # [128, 2, 1024]: [:, :, 0:512] = A-fields' P|Q cols.
                PQv = PQ[:].rearrange("p (a b) -> p a b", a=2)
                Gm = gmp.tile([128, 4 * NPIX], dt.bfloat16, tag="Gm")
                for fh in range(2):
                    sig = sigp.tile([128, 8 * NPIX], dt.bfloat16, tag="sig")
                    sigv = sig[:, 0:1024].rearrange("p (a b) -> p a b", a=2)
                    nc.scalar.activation(
                        sigv, PQv[:, :, fh * 512:(fh + 1) * 512],
                        AF.Sigmoid, 0.0, K)
                    nc.vector.tensor_tensor(
                        Gm[:, fh * 512:(fh + 1) * 512],
                        sig[:, 0:512], sig[:, 512:1024], op.mult)
            Fp = fpp.tile([128, 2 * NPIX], dt.bfloat16, tag="Fp")
            nc.vector.scalar_tensor_tensor(
                Fp[:], Gm[:, 0:512], 1.0, Gm[:, 512:1024], op.mult, op.mult,
                accum_out=Ssum[:, u:u + 1])
            F12 = fpp.tile([128, NPIX], dt.bfloat16, tag="F12")
            nc.vector.scalar_tensor_tensor(
                F12[:], Fp[:, 0:NPIX], 1.0, Fp[:, NPIX:2 * NPIX], op.mult, op.mult,
                accum_out=Isum[:, u:u + 1])
            if u == U1 - 1 and U1 < U:
                nc.sync.dma_start(out=OUTv[:, :, 0:U1], in_=SI[:, :, 0:U1])

        if U1 < U:
            nc.sync.dma_start(out=OUTv[:, :, U1:U], in_=SI[:, :, U1:U])
        else:
            nc.sync.dma_start(out=OUTv[:], in_=SI[:])

    nc.finalize()
    return nc


def _get_compiled(U):
    key = ("nc", U)
    if key not in _cache:
        _cache[key] = _build_nc(U)
    return _cache[key]


def _get_runner(U):
    """Persistent jitted shard_map callable (cached per unit count U)."""
    key = ("runner", U)
    if key in _cache:
        return _cache[key]

    import jax
    import numpy as _np
    from jax.experimental.shard_map import shard_map
    from jax.sharding import Mesh, PartitionSpec

    import concourse.bass2jax as b2j
    from concourse import mybir

    nc = _get_compiled(U)
    b2j.install_neuronx_cc_hook()
    partition_name = nc.partition_id_tensor.name if nc.partition_id_tensor else None

    in_names, out_names, out_avals, zero_shapes = [], [], [], []
    for alloc in nc.m.functions[0].allocations:
        if not isinstance(alloc, mybir.MemoryLocationSet):
            continue
        name = alloc.memorylocations[0].name
        if alloc.kind == "ExternalInput":
            if name != partition_name:
                in_names.append(name)
        elif alloc.kind == "ExternalOutput":
            out_names.append(name)
            shape = tuple(alloc.tensor_shape)
            dtype = mybir.dt.np(alloc.dtype)
            out_avals.append(jax.core.ShapedArray(shape, dtype))
            zero_shapes.append((shape, dtype))
    n_params = len(in_names)
    n_outs = len(out_avals)
    all_names = list(in_names) + list(out_names)
    if partition_name is not None:
        all_names.append(partition_name)
    donate = tuple(range(n_params, n_params + n_outs))

    def _body(*args):
        operands = list(args)
        if partition_name is not None:
            operands.append(b2j.partition_id_tensor())
        outs = b2j._bass_exec_p.bind(
            *operands,
            out_avals=tuple(out_avals),
            in_names=tuple(all_names),
            out_names=tuple(out_names),
            lowering_input_output_aliases=(),
            sim_require_finite=True,
            sim_require_nnan=True,
            nc=nc,
        )
        return tuple(outs)

    devices = jax.devices()[:NC]
    assert len(devices) >= NC, f"need {NC} devices, have {len(jax.devices())}"
    mesh = Mesh(_np.asarray(devices), ("core",))
    in_specs = (PartitionSpec("core"),) * (n_params + n_outs)
    out_specs = (PartitionSpec("core"),) * n_outs
    sharded = jax.jit(
        shard_map(_body, mesh=mesh, in_specs=in_specs, out_specs=out_specs,
                  check_rep=False),
        donate_argnums=donate,
        keep_unused=True,
    )

    def run(in_maps):
        concat_in = [
            np.concatenate([np.asarray(in_maps[c][nm]) for c in range(NC)], axis=0)
            for nm in in_names
        ]
        zeros = [np.zeros((NC * sh[0], *sh[1:]), dtp) for sh, dtp in zero_shapes]
        out_arrs = sharded(*concat_in, *zeros)
        return [
            {nm: np.asarray(out_arrs[i]).reshape(NC, *out_avals[i].shape)[c]
             for i, nm in enumerate(out_names)}
            for c in range(NC)
        ]

    _cache[key] = run
    return run


def kernel(loc_p, loc_t, grid):
    assert int(grid) == G
    loc_p = np.asarray(loc_p, dtype=np.float32)
    loc_t = np.asarray(loc_t, dtype=np.float32)
    n_p, n_t = loc_p.shape[0], loc_t.shape[0]

    P = _derived(loc_p)
    T = _derived(loc_t)

    # Pairs whose DELTA-dilated AABBs overlap; everything else is < 1e-14.
    ox = (P["x0"][:, None] <= T["x1"][None, :] + DELTA) & \
         (T["x0"][None, :] <= P["x1"][:, None] + DELTA)
    oy = (P["y0"][:, None] <= T["y1"][None, :] + DELTA) & \
         (T["y0"][None, :] <= P["y1"][:, None] + DELTA)
    idx = np.argwhere(ox & oy)
    if len(idx):
        idx = idx[~_sat_separated(P, T, idx[:, 0], idx[:, 1], float(DELTA))]

    # Round-robin pairs over cores; pad each core to U*128 with dummy pairs.
    per_core = [idx[c::NC] for c in range(NC)]
    U = max(1, -(-max(len(pc) for pc in per_core) // 128))
    # Shave one device unit per core by computing the tail spill (at most
    # 8*128 pairs) exactly on the host.
    spill_pairs = None
    if U > 1:
        cap = (U - 1) * 128
        spill_pairs = np.concatenate(
            [pc[cap:] for pc in per_core if len(pc) > cap], axis=0)
        per_core = [pc[:cap] for pc in per_core]
        U = U - 1

    basis = _basis()
    UR = max(U - 1, 1)
    in_maps = []
    for c in range(NC):
        pc = per_core[c]
        lh = np.zeros((24, U * 128), dtype=np.float32)
        if len(pc):
            lh[:, :len(pc)] = _pair_coeffs(P, T, pc[:, 0], pc[:, 1])
        lh = lh.astype(np.float16)
        ft = np.empty((12, 256 + 4 * NPIX), dtype=np.float16)
        ft[:, 0:128] = lh[:12, 0:128]
        ft[:, 128:256] = lh[12:, 0:128]
        ft[:, 256:] = basis
        lhpr = np.zeros((12, UR * 128), dtype=np.float16)
        lhqr = np.zeros((12, UR * 128), dtype=np.float16)
        if U > 1:
            lhpr[:] = lh[:12, 128:]
            lhqr[:] = lh[12:, 128:]
        in_maps.append({"FT": ft, "LHPR": lhpr, "LHQR": lhqr})

    try:
        res = _get_runner(U)(in_maps)
    except Exception:
        # Robust fallback: the stock (slower) dispatch path.
        from concourse.bass_utils import run_bass_kernel_spmd

        res = run_bass_kernel_spmd(
            _get_compiled(U), in_maps, core_ids=list(range(NC))).results

    out = np.zeros((n_p, n_t), dtype=np.float32)
    for c in range(NC):
        pc = per_core[c]
        if len(pc):
            si = res[c]["OUT"]  # [128, 2U]: S cols then I cols
            S = si[:, :U].T.reshape(-1)[:len(pc)]
            I = si[:, U:].T.reshape(-1)[:len(pc)]
            out[pc[:, 0], pc[:, 1]] = I / (S - I + EPS)
    if spill_pairs is not None and len(spill_pairs):
        out[spill_pairs[:, 0], spill_pairs[:, 1]] = _host_exact(
            loc_p, loc_t, spill_pairs)
    return out


# revision 39
# speedup vs baseline: 1.0059x; 1.0059x over previous
"""Sparse PIoU (pixel-wise IoU) pairwise matrix kernel for Trainium2, 8 cores.

Math: for each pair (pred box n, target box m) the reference samples a 16x16
grid of the joint AABB and evaluates soft memberships
F = sigmoid(k(w/2-|A|)) * sigmoid(k(h/2-|B|)) per box, with (A, B) the pixel
offsets rotated into the box frame.  A and B are affine in the grid coords
(ug, uh), so the sigmoid args P = s/2 - A and Q = s/2 + A for all 256 pixels
x 4 fields come from K=12 matmuls against a tiny constant basis (1, ug, uh
per field); since P + Q = s >= 8 and k = 10, sigmoid(kP)*sigmoid(kQ) ==
sigmoid(k*min(P,Q)) to machine precision, giving the |.| for free.

Sparsity: boxes are small (8..96 px) in a 640x640 field, so only ~7% of the
512x512 pairs have margin-dilated AABB overlap AND pass a 2px-margin
separating-axis test; every excluded pair has true PIoU < 1e-12 (sigmoid
tails at >= 2px separation).  The host computes both filters, round-robins
surviving pairs over the 8 cores packed one-per-partition into units of 128
(~17 units/core vs the dense kernel's 256), and scatters device results
into a zero matrix.  The sub-unit tail spill (< 128 pairs/core) is computed
exactly on the host so the device runs one unit fewer.

Device pipeline per unit u (128 pairs):
    PE  : 4 matmuls [12,128]x[12,512] -> PSUM [128, 2048] = P|Q sig args
          (P and Q are two weight-sets against the same shared basis)
    ACT : sig = Sigmoid(K * PQ)            [128, 2048] bf16 (one instruction)
    DVE : Gm  = sigP * sigQ                [128, 1024] (field memberships)
    DVE : Fp  = gA * gB,  accum -> S[u]    (fused product+reduce, STT)
    DVE : F12 = F1 * F2,  accum -> I[u]    (fused product+reduce, STT)
The raw S|I accumulators stream out by DMA (all-but-last unit early, last
unit at the end); the tiny piou = I/(S-I+eps) division happens on the host.
Measured: the ACT sigmoid stream (1.97us/unit, zero gaps) and the DVE chain
(1.97us/unit) are co-paced bottlenecks; PE units take 1.7us so matmuls stay
ahead; the last unit's sigmoid runs in two field-halves to shorten the
closing DVE chain.  ~24us of the span is fixed NEFF preamble/postamble.

Host precomputes the per-pair coefficient slab directly in the transposed
fp16 layout the PE wants (lhsT [24, pairs]), eliminating the on-device
coefficient phase, PE transposes and stash copies of the dense kernel.  The
unit-0 coefficients ride in one combined tensor with the basis so a single
early DMA unblocks the first matmuls.

Dispatch uses a persistent jitted shard_map callable (cached per unit-count
U, recompiled automatically if denser inputs need more units) so
steady-state calls skip jax re-trace/re-lowering.
"""

import numpy as np

N = 512
M = 512
G = 16
NPIX = G * G
K_SLOPE = np.float32(10.0)
EPS = np.float32(1e-6)
NC = 8
DELTA = np.float32(2.0)  # separation margin in px (excluded-pair PIoU < 1e-12)

_cache = {}


def _derived(b):
    # b: [K,5] float32 -> per-box derived quantities (all float32)
    cx, cy, w, h, t = (b[:, i].astype(np.float32) for i in range(5))
    c, s = np.cos(t).astype(np.float32), np.sin(t).astype(np.float32)
    hw = np.float32(0.5) * (w * np.abs(c) + h * np.abs(s))
    hh = np.float32(0.5) * (w * np.abs(s) + h * np.abs(c))
    return dict(
        cx=cx, cy=cy, ct=c, st=s,
        shw=np.float32(0.5) * w, shh=np.float32(0.5) * h,
        x0=cx - hw, x1=cx + hw, y0=cy - hh, y1=cy + hh,
    )


def _basis():
    # [12, 1024] fp16 (values exact): field f at cols f*256..(f+1)*256 uses
    # rows 3f..3f+2 = (1, Ug, Uh).  Pixel p = h*G+g -> Ug[p]=u[g], Uh[p]=u[h].
    # The P and Q sigmoid-arg blocks share this basis (their coefficients are
    # two weight-sets against the same moving tensor).
    u = (np.arange(G, dtype=np.float32) + np.float32(0.5)) / np.float32(G)
    Ug = np.tile(u, G)
    Uh = np.repeat(u, G)
    bas = np.zeros((12, 4 * NPIX), dtype=np.float32)
    for f in range(4):
        c0 = f * NPIX
        bas[3 * f + 0, c0:c0 + NPIX] = 1.0
        bas[3 * f + 1, c0:c0 + NPIX] = Ug
        bas[3 * f + 2, c0:c0 + NPIX] = Uh
    return bas.astype(np.float16)  # (2g+1)/32 values are exact in fp16


def _sat_separated(P, T, n_idx, m_idx, margin):
    """True for pairs whose margin-dilated rotated boxes are disjoint
    (separating-axis test on the 4 edge normals).  At k=10, a separation
    margin of 2px bounds the true PIoU of excluded pairs below ~1e-12."""
    dcx = T["cx"][m_idx] - P["cx"][n_idx]
    dcy = T["cy"][m_idx] - P["cy"][n_idx]
    sep = np.zeros(n_idx.size, dtype=bool)
    for src in (0, 1):
        B1, i1 = (P, n_idx) if src == 0 else (T, m_idx)
        B2, i2 = (T, m_idx) if src == 0 else (P, n_idx)
        ct, st = B1["ct"][i1], B1["st"][i1]
        c2, s2 = B2["ct"][i2], B2["st"][i2]
        for ax in range(2):
            ux, uy = (ct, st) if ax == 0 else (-st, ct)
            e1 = B1["shw" if ax == 0 else "shh"][i1]
            e2 = (B2["shw"][i2] * np.abs(ux * c2 + uy * s2)
                  + B2["shh"][i2] * np.abs(-ux * s2 + uy * c2))
            sep |= np.abs(ux * dcx + uy * dcy) > e1 + e2 + margin
    return sep


def _host_exact(loc_p, loc_t, pairs):
    """Exact reference math (vectorized numpy) for a small list of pairs —
    used for the tail spill so the device runs one unit fewer per core."""
    b1 = loc_p[pairs[:, 0]]
    b2 = loc_t[pairs[:, 1]]
    u = (np.arange(G, dtype=np.float32) + np.float32(0.5)) / np.float32(G)

    def aabb(b):
        cx, cy, w, h, t = (b[:, i] for i in range(5))
        c, s = np.abs(np.cos(t)), np.abs(np.sin(t))
        hw = 0.5 * (w * c + h * s)
        hh = 0.5 * (w * s + h * c)
        return cx - hw, cx + hw, cy - hh, cy + hh

    x0p, x1p, y0p, y1p = aabb(b1)
    x0t, x1t, y0t, y1t = aabb(b2)
    xmin = np.minimum(x0p, x0t)
    xmax = np.maximum(x1p, x1t)
    ymin = np.minimum(y0p, y0t)
    ymax = np.maximum(y1p, y1t)
    px = xmin[:, None] + (xmax - xmin)[:, None] * u[None, :]  # [k,G]
    py = ymin[:, None] + (ymax - ymin)[:, None] * u[None, :]
    PX = px[:, None, :]  # [k,1,G] broadcast over rows
    PY = py[:, :, None]  # [k,G,1]

    def member(b):
        cx, cy, w, h, t = (b[:, i, None, None] for i in range(5))
        ct, st = np.cos(t), np.sin(t)
        dx = PX - cx
        dy = PY - cy
        dw = np.abs(dx * ct + dy * st)
        dh = np.abs(-dx * st + dy * ct)
        def sig(z):
            with np.errstate(over="ignore"):  # exp overflow -> sig == 0.0
                return 1.0 / (1.0 + np.exp(-z))

        return (sig(float(K_SLOPE) * (0.5 * w - dw))
                * sig(float(K_SLOPE) * (0.5 * h - dh)))

    F1 = member(b1)
    F2 = member(b2)
    inter = np.sum(F1 * F2, axis=(1, 2))
    union = np.sum(F1, axis=(1, 2)) + np.sum(F2, axis=(1, 2)) - inter
    return (inter / (union + EPS)).astype(np.float32)


def _pair_coeffs(P, T, n_idx, m_idx):
    """[24, npairs] float32 coefficient slab for the given (n, m) pairs.

    Row 3f+r holds the P-arg coeff of field f on basis fn r in (1, ug, uh);
    row 12+3f+r the Q-arg coeff.  Field order: A-pred, A-targ, B-pred, B-targ
    so that Gm pairs A1|A2 with B1|B2 and Fp pairs F1 with F2 downstream.
    """
    p = {k: v[n_idx] for k, v in P.items()}
    t = {k: v[m_idx] for k, v in T.items()}
    xmin = np.minimum(p["x0"], t["x0"])
    xmax = np.maximum(p["x1"], t["x1"])
    ymin = np.minimum(p["y0"], t["y0"])
    ymax = np.maximum(p["y1"], t["y1"])
    sx = xmax - xmin
    sy = ymax - ymin
    C = np.empty((24, n_idx.size), dtype=np.float32)
    for f, (b, ab) in enumerate(((p, "a"), (t, "a"), (p, "b"), (t, "b"))):
        dx0 = xmin - b["cx"]
        dy0 = ymin - b["cy"]
        if ab == "a":
            c0 = dx0 * b["ct"] + dy0 * b["st"]
            c1 = sx * b["ct"]
            c2 = sy * b["st"]
            half = b["shw"]
        else:
            c0 = dy0 * b["ct"] - dx0 * b["st"]
            c1 = -sx * b["st"]
            c2 = sy * b["ct"]
            half = b["shh"]
        C[3 * f + 0] = half - c0
        C[3 * f + 1] = -c1
        C[3 * f + 2] = -c2
        C[12 + 3 * f + 0] = half + c0
        C[12 + 3 * f + 1] = c1
        C[12 + 3 * f + 2] = c2
    return C


def _build_nc(U):
    from contextlib import ExitStack

    import concourse.bacc as bacc
    import concourse.tile as tile
    from concourse import mybir

    dt = mybir.dt
    op = mybir.AluOpType
    AF = mybir.ActivationFunctionType
    K = float(K_SLOPE)

    nc = bacc.Bacc(None, target_bir_lowering=False)
    # FT packs [unit-0 P-coeffs | unit-0 Q-coeffs | basis] so one fast DMA
    # delivers everything the first unit's matmuls need.
    FT_d = nc.declare_dram_parameter("FT", [12, 256 + 4 * NPIX], dt.float16, isOutput=False)
    UR = max(U - 1, 1)
    LHPR_d = nc.declare_dram_parameter("LHPR", [12, UR * 128], dt.float16, isOutput=False)
    LHQR_d = nc.declare_dram_parameter("LHQR", [12, UR * 128], dt.float16, isOutput=False)
    # Raw S|I accumulators; the tiny piou = I/(S-I+eps) division happens on
    # the host, which keeps the device tail to just the last STT + DMA.
    OUT_d = nc.declare_dram_parameter("OUT", [128, 2 * U], dt.float32, isOutput=True)

    with tile.TileContext(nc) as tc, ExitStack() as ctx:
        consts = ctx.enter_context(tc.tile_pool(name="consts", bufs=1))
        sigp = ctx.enter_context(tc.tile_pool(name="sigp", bufs=4))
        gmp = ctx.enter_context(tc.tile_pool(name="gmp", bufs=3))
        fpp = ctx.enter_context(tc.tile_pool(name="fpp", bufs=3))
        accp = ctx.enter_context(tc.tile_pool(name="accp", bufs=1))
        psum = ctx.enter_context(tc.tile_pool(name="psum", bufs=2, space="PSUM"))

        # Warm the PE clock on a memset scratch tile: no DMA dependency, so
        # the ramp starts right after the preamble barrier.  The memset runs
        # on the (otherwise idle) vector queue so the gpsimd/sync queues get
        # their DMA triggers out without delay.
        Wz = consts.tile([12, 128], dt.bfloat16)
        nc.vector.memset(Wz[:], 0.0)
        Tw = psum.tile([128, 8 * NPIX], dt.float32, tag="pq")
        nc.tensor.matmul(
            Tw[:, 0:128], Wz[:], Wz[:], start=True, stop=True)

        # Input DMAs: all on the sync queue (hardware DGE; gpsimd triggers go
        # through the slow SWDGE path, and a scalar-queue trigger makes
        # walrus re-emit the sigmoid ACT_TABLE_LOAD).  FT (unit-0 coeffs +
        # basis, ~31KB) first so unit 0 starts ~1.5us before the bulk slabs
        # finish landing.
        FT = consts.tile([12, 256 + 4 * NPIX], dt.float16)
        LHPR = consts.tile([12, UR, 128], dt.float16)
        LHQR = consts.tile([12, UR, 128], dt.float16)
        nc.sync.dma_start(out=FT[:], in_=FT_d[:])
        nc.sync.dma_start(
            out=LHPR[:].rearrange("p a b -> p (a b)"), in_=LHPR_d[:])
        nc.sync.dma_start(
            out=LHQR[:].rearrange("p a b -> p (a b)"), in_=LHQR_d[:])
        BAS = FT[:, 256:256 + 4 * NPIX]

        SI = accp.tile([128, 2, U], dt.float32)
        Ssum = SI[:, 0, :]
        Isum = SI[:, 1, :]
        OUTv = OUT_d[:].rearrange("p (a b) -> p a b", a=2)

        # S|I columns DMA out in two chunks: all but the last unit while the
        # ACT stream is still running, the final column right at the end.
        U1 = U - 1 if U > 1 else U

        for u in range(U):
            PQ = psum.tile([128, 8 * NPIX], dt.float32, tag="pq")
            for h in range(4):
                if u == 0:
                    lhsT = FT[:, 0:128] if h < 2 else FT[:, 128:256]
                else:
                    lhsT = (LHPR if h < 2 else LHQR)[:, u - 1, :]
                nc.tensor.matmul(
                    PQ[:, h * 512:(h + 1) * 512],
                    lhsT,
                    BAS[:, (h % 2) * 512:(h % 2 + 1) * 512],
                    start=True, stop=True)
            last = u == U - 1 and U > 1
            if not last:
                sig = sigp.tile([128, 8 * NPIX], dt.bfloat16, tag="sig")
                nc.scalar.activation(sig[:], PQ[:], AF.Sigmoid, 0.0, K)
                Gm = gmp.tile([128, 4 * NPIX], dt.bfloat16, tag="Gm")
                nc.vector.tensor_tensor(
                    Gm[:], sig[:, 0:1024], sig[:, 1024:2048], op.mult)
            else:
                # Final unit: sigmoid in two field-halves (A then B) so the
                # closing DVE chain starts one ACT-half earlier.  PQ viewed
                # [128, 2, 1024]: [:, :, 0:512] = A-fields' P|Q cols.
                PQv = PQ[:].rearrange("p (a b) -> p a b", a=2)
                Gm = gmp.tile([128, 4 * NPIX], dt.bfloat16, tag="Gm")
                for fh in range(2):
                    sig = sigp.tile([128, 8 * NPIX], dt.bfloat16, tag="sig")
                    sigv = sig[:, 0:1024].rearrange("p (a b) -> p a b", a=2)
                    nc.scalar.activation(
                        sigv, PQv[:, :, fh * 512:(fh + 1) * 512],
                        AF.Sigmoid, 0.0, K)
                    nc.vector.tensor_tensor(
                        Gm[:, fh * 512:(fh + 1) * 512],
                        sig[:, 0:512], sig[:, 512:1024], op.mult)
            Fp = fpp.tile([128, 2 * NPIX], dt.bfloat16, tag="Fp")
            nc.vector.scalar_tensor_tensor(
                Fp[:], Gm[:, 0:512], 1.0, Gm[:, 512:1024], op.mult, op.mult,
                accum_out=Ssum[:, u:u + 1])
            F12 = fpp.tile([128, NPIX], dt.bfloat16, tag="F12")
            nc.vector.scalar_tensor_tensor(
                F12[:], Fp[:, 0:NPIX], 1.0, Fp[:, NPIX:2 * NPIX], op.mult, op.mult,
                accum_out=Isum[:, u:u + 1])
            if u == U1 - 1 and U1 < U:
                nc.sync.dma_start(out=OUTv[:, :, 0:U1], in_=SI[:, :, 0:U1])

        if U1 < U:
            nc.sync.dma_start(out=OUTv[:, :, U1:U], in_=SI[:, :, U1:U])
        else:
            nc.sync.dma_start(out=OUTv[:], in_=SI[:])

    nc.finalize()
    return nc


def _get_compiled(U):
    key = ("nc", U)
    if key not in _cache:
        _cache[key] = _build_nc(U)
    return _cache[key]


def _get_runner(U):
    """Persistent jitted shard_map callable (cached per unit count U)."""
    key = ("runner", U)
    if key in _cache:
        return _cache[key]

    import jax
    import numpy as _np
    from jax.experimental.shard_map import shard_map
    from jax.sharding import Mesh, PartitionSpec

    import concourse.bass2jax as b2j
    from concourse import mybir

    nc = _get_compiled(U)
    b2j.install_neuronx_cc_hook()
    partition_name = nc.partition_id_tensor.name if nc.partition_id_tensor else None

    in_names, out_names, out_avals, zero_shapes = [], [], [], []
    for alloc in nc.m.functions[0].allocations:
        if not isinstance(alloc, mybir.MemoryLocationSet):
            continue
        name = alloc.memorylocations[0].name
        if alloc.kind == "ExternalInput":
            if name != partition_name:
                in_names.append(name)
        elif alloc.kind == "ExternalOutput":
            out_names.append(name)
            shape = tuple(alloc.tensor_shape)
            dtype = mybir.dt.np(alloc.dtype)
            out_avals.append(jax.core.ShapedArray(shape, dtype))
            zero_shapes.append((shape, dtype))
    n_params = len(in_names)
    n_outs = len(out_avals)
    all_names = list(in_names) + list(out_names)
    if partition_name is not None:
        all_names.append(partition_name)
    donate = tuple(range(n_params, n_params + n_outs))

    def _body(*args):
        operands = list(args)
        if partition_name is not None:
            operands.append(b2j.partition_id_tensor())
        outs = b2j._bass_exec_p.bind(
            *operands,
            out_avals=tuple(out_avals),
            in_names=tuple(all_names),
            out_names=tuple(out_names),
            lowering_input_output_aliases=(),
            sim_require_finite=True,
            sim_require_nnan=True,
            nc=nc,
        )
        return tuple(outs)

    devices = jax.devices()[:NC]
    assert len(devices) >= NC, f"need {NC} devices, have {len(jax.devices())}"
    mesh = Mesh(_np.asarray(devices), ("core",))
    in_specs = (PartitionSpec("core"),) * (n_params + n_outs)
    out_specs = (PartitionSpec("core"),) * n_outs
    sharded = jax.jit(
        shard_map(_body, mesh=mesh, in_specs=in_specs, out_specs=out_specs,
                  check_rep=False),
        donate_argnums=donate,
        keep_unused=True,
    )

    def run(in_maps):
        concat_in = [
            np.concatenate([np.asarray(in_maps[c][nm]) for c in range(NC)], axis=0)
            for nm in in_names
        ]
        zeros = [np.zeros((NC * sh[0], *sh[1:]), dtp) for sh, dtp in zero_shapes]
        out_arrs = sharded(*concat_in, *zeros)
        return [
            {nm: np.asarray(out_arrs[i]).reshape(NC, *out_avals[i].shape)[c]
             for i, nm in enumerate(out_names)}
            for c in range(NC)
        ]

    _cache[key] = run
    return run


def kernel(loc_p, loc_t, grid):
    assert int(grid) == G
    loc_p = np.asarray(loc_p, dtype=np.float32)
    loc_t = np.asarray(loc_t, dtype=np.float32)
    n_p, n_t = loc_p.shape[0], loc_t.shape[0]

    P = _derived(loc_p)
    T = _derived(loc_t)

    # Pairs whose DELTA-dilated AABBs overlap; everything else is < 1e-14.
    ox = (P["x0"][:, None] <= T["x1"][None, :] + DELTA) & \
         (T["x0"][None, :] <= P["x1"][:, None] + DELTA)
    oy = (P["y0"][:, None] <= T["y1"][None, :] + DELTA) & \
         (T["y0"][None, :] <= P["y1"][:, None] + DELTA)
    idx = np.argwhere(ox & oy)
    if len(idx):
        idx = idx[~_sat_separated(P, T, idx[:, 0], idx[:, 1], float(DELTA))]

    # Round-robin pairs over cores; pad each core to U*128 with dummy pairs.
    per_core = [idx[c::NC] for c in range(NC)]
    U = max(1, -(-max(len(pc) for pc in per_core) // 128))
    # Shave one device unit per core by computing the tail spill (at most
    # 8*128 pairs) exactly on the host.
    spill_pairs = None
    if U > 1:
        cap = (U - 1) * 128
        spill_pairs = np.concatenate(
            [pc[cap:] for pc in per_core if len(pc) > cap], axis=0)
        per_core = [pc[:cap] for pc in per_core]
        U = U - 1

    basis = _basis()
    UR = max(U - 1, 1)
    in_maps = []
    for c in range(NC):
        pc = per_core[c]
        lh = np.zeros((24, U * 128), dtype=np.float32)
        if len(pc):
            lh[:, :len(pc)] = _pair_coeffs(P, T, pc[:, 0], pc[:, 1])
        lh = lh.astype(np.float16)
        ft = np.empty((12, 256 + 4 * NPIX), dtype=np.float16)
        ft[:, 0:128] = lh[:12, 0:128]
        ft[:, 128:256] = lh[12:, 0:128]
        ft[:, 256:] = basis
        lhpr = np.zeros((12, UR * 128), dtype=np.float16)
        lhqr = np.zeros((12, UR * 128), dtype=np.float16)
        if U > 1:
            lhpr[:] = lh[:12, 128:]
            lhqr[:] = lh[12:, 128:]
        in_maps.append({"FT": ft, "LHPR": lhpr, "LHQR": lhqr})

    try:
        res = _get_runner(U)(in_maps)
    except Exception:
        # Robust fallback: the stock (slower) dispatch path.
        from concourse.bass_utils import run_bass_kernel_spmd

        res = run_bass_kernel_spmd(
            _get_compiled(U), in_maps, core_ids=list(range(NC))).results

    out = np.zeros((n_p, n_t), dtype=np.float32)
    for c in range(NC):
        pc = per_core[c]
        if len(pc):
            si = res[c]["OUT"]  # [128, 2U]: S cols then I cols
            S = si[:, :U].T.reshape(-1)[:len(pc)]
            I = si[:, U:].T.reshape(-1)[:len(pc)]
            out[pc[:, 0], pc[:, 1]] = I / (S - I + EPS)
    if spill_pairs is not None and len(spill_pairs):
        out[spill_pairs[:, 0], spill_pairs[:, 1]] = _host_exact(
            loc_p, loc_t, spill_pairs)
    return out
